# revision 53
# baseline (speedup 1.0000x reference)
"""GCN block (GCNConv + LayerNorm + ReLU) on 8 Trainium2 NeuronCores.

Strategy:
  - out = LN(A_norm @ x @ W^T + b) with A_norm = D^-1/2 A D^-1/2 (self-loops
    included).  LayerNorm is scale-invariant per row, so the dst-side scaling
    dinv[dst] can be dropped entirely if the bias is pre-scaled per row:
    LN(dinv_d * (A_d + sqrt(deg_d) * b)) == LN(A_d + sqrt(deg_d) * b), where
    A_d = sum_{e->d} dinv[src] x[src] + dinv[d] x[d].
  - dinv[src] is folded into the gather table on the host (xs = dinv * x,
    bf16), which makes every scatter matrix PURELY BINARY -> precomputed on
    the host and streamed to SBUF as fp8 (0/1 exact): no on-device scatter
    matrix construction at all.
  - Destination nodes sharded contiguously across 8 cores (6250 rows each,
    49 blocks of 128).  Edges bucketed per (dst-block, src-parity table) and
    padded to 128-edge tiles; source rows fetched with dma_gather (bf16,
    even/odd tables so row indices fit int16; 1024 idxs per call is a HW
    limit; descriptor generation is serial on the gpsimd engine and is this
    kernel's bottleneck at ~2.7 ns/row).
  - Per 128-edge tile: aggT += G^T @ S accumulated in PSUM ([ch, dst]); the
    self-loop contribution comes from a contiguous shard load used as lhsT
    against a shared identity tile (keeps 6250 rows/core out of the random
    gather).  aggT @ W^T plus a rank-1 bias matmul (sqrtdeg x [b|sum b])
    yields po = A + sqrt(deg) b with a free row-sum column for the LN mean.
  - Epilogue: Square+accum (ACT) for ssq, small DVE ops for mu/var/rstd,
    one fused ACT Relu(po * rstd + (-mu * rstd)) pass, bf16 store (cast to
    f32 on the host).
  - Per-chunk byte-packed const streams (idx+S+xsh in ONE dma each): each
    dma_start costs ~0.65us of dispatch time on its issuing engine, and a
    coarse const tile stalls every consumer until its whole dma lands, so
    few-but-chunk-aligned loads are critical.  Out-stores ride the sync
    engine after the ~30 load dispatches are done; the first two chunks are
    single blocks so the pipeline fills early.
  - Emission is software-pipelined: block b's aggregation matmuls are
    emitted before block b-1's W-matmul so the tensor engine never waits
    on the PSUM->SBUF copies.
"""

import math
import sys

sys.path.insert(0, "/opt/trn_rl_repo")

import numpy as np
import ml_dtypes

N_NODES = 50000
WIDTH = 256
N_CORES = 8
NODES_PER_CORE = N_NODES // N_CORES  # 6250
P = 128
N_BLOCKS = math.ceil(NODES_PER_CORE / P)  # 49 (last block has 106 rows)
LN_EPS = 1e-5
HALF = N_NODES // 2  # rows per gather table

GATHER_TILE_CAP = 8  # max tiles (128 idxs each) per dma_gather call (HW limit 1024 idxs)

_f8np = ml_dtypes.float8_e4m3
_bfnp = ml_dtypes.bfloat16


def _preprocess(edge_index):
    """Bucket non-self-loop edges by (core, dst-block, src-parity table), pad
    to whole 128-edge tiles, and build the binary scatter-tile stream.

    Per-block processing tile order: even-table tiles, odd-table tiles, then
    one self-loop identity tile (lhsT = contiguous shard rows).
    """
    src = np.asarray(edge_index[0]).astype(np.int64)
    dst = np.asarray(edge_index[1]).astype(np.int64)

    deg = np.bincount(dst, minlength=N_NODES).astype(np.float64) + 1.0  # + self
    dinv = 1.0 / np.sqrt(deg)
    sqdeg_all = np.sqrt(deg)

    core = dst // NODES_PER_CORE
    r = dst % NODES_PER_CORE
    blk = np.minimum(r // P, N_BLOCKS - 1)
    dcol = r - blk * P
    tab = src & 1
    gbin = (core * N_BLOCKS + blk) * 2 + tab

    order = np.argsort(gbin, kind="stable")
    src_s, dcol_s, gbin_s = src[order], dcol[order], gbin[order]

    cnt = np.bincount(gbin, minlength=N_CORES * N_BLOCKS * 2).reshape(
        N_CORES, N_BLOCKS, 2
    )
    TL = [int(math.ceil(int(cnt[:, b, 0].max()) / P)) for b in range(N_BLOCKS)]
    TH = [int(math.ceil(int(cnt[:, b, 1].max()) / P)) for b in range(N_BLOCKS)]
    sTL, sTH = sum(TL), sum(TH)
    stot = sTL + sTH
    EOFF = np.concatenate([[0], np.cumsum(TL)])  # even gather tile offsets
    OOFF = np.concatenate([[0], np.cumsum(TH)])  # odd gather tile offsets
    TOFF = np.concatenate(
        [[0], np.cumsum(np.asarray(TL) + np.asarray(TH))]
    )  # S-stream tile offsets (self tiles are shared identity tiles)

    starts = np.concatenate([[0], np.cumsum(cnt.ravel())])[:-1]
    j = np.arange(len(gbin_s)) - starts[gbin_s]  # index within bucket
    c = gbin_s // (N_BLOCKS * 2)
    b = (gbin_s // 2) % N_BLOCKS
    t = gbin_s & 1
    til = j // P
    p = j % P

    # gather index arrays (per-table tile order)
    idxe_flat = np.zeros((N_CORES, sTL * P), np.int16)
    idxo_flat = np.zeros((N_CORES, sTH * P), np.int16)
    idx16 = (src_s >> 1).astype(np.int16)
    Je = (EOFF[b] + til) * P + p
    Jo = (OOFF[b] + til) * P + p
    ev = t == 0
    idxe_flat[c[ev], Je[ev]] = idx16[ev]
    idxo_flat[c[~ev], Jo[~ev]] = idx16[~ev]

    # binary S stream: [core, P, stot*P] fp8
    S_all = np.zeros((N_CORES, P, stot * P), _f8np)
    tg = np.where(t == 0, TOFF[b] + til, TOFF[b] + np.asarray(TL)[b] + til)
    S_all[c, p, tg * P + dcol_s] = _f8np(1.0)

    # wrap: flat j -> (partition j%16, column j//16), replicated on 8 stripes
    def wrap(flat, ntiles):
        if ntiles == 0:
            return np.zeros((N_CORES, P, 0), np.int16)
        a = flat.reshape(N_CORES, ntiles * 8, 16).transpose(0, 2, 1)
        return np.ascontiguousarray(np.tile(a, (1, 8, 1)))  # [8,128,cols]

    return (
        TL,
        TH,
        dinv,
        sqdeg_all,
        S_all,
        wrap(idxe_flat, sTL),
        wrap(idxo_flat, sTH),
    )


def _chunks(TL, TH):
    """Group consecutive blocks into gather chunks where EACH table's tile
    count stays within one dma_gather call's limit.  The first two chunks are
    single blocks so the pipeline starts as early as possible."""
    out = []
    cur = []
    ne = no = 0
    for b in range(N_BLOCKS):
        if cur and (
            len(out) < 2
            or ne + TL[b] > GATHER_TILE_CAP
            or no + TH[b] > GATHER_TILE_CAP
        ):
            out.append((cur, ne, no))
            cur, ne, no = [], 0, 0
        cur.append(b)
        ne += TL[b]
        no += TH[b]
    if cur:
        out.append((cur, ne, no))
    return out


def _build_program(TL, TH, generic_affine):
    import concourse.bass as bass
    import concourse.tile as tile
    from concourse import bacc as bacc_mod
    from concourse import mybir
    from contextlib import ExitStack

    f32 = mybir.dt.float32
    bf16 = mybir.dt.bfloat16
    f8 = mybir.dt.float8e4
    i16 = mybir.dt.int16
    Alu = mybir.AluOpType
    Act = mybir.ActivationFunctionType

    sTL, sTH = sum(TL), sum(TH)
    EOFF = np.concatenate([[0], np.cumsum(TL)])
    OOFF = np.concatenate([[0], np.cumsum(TH)])
    TOFF = np.concatenate([[0], np.cumsum(np.asarray(TL) + np.asarray(TH))])
    chunks = _chunks(TL, TH)

    nc = bacc_mod.Bacc(None, target_bir_lowering=False, debug=False, num_swdge_queues=4)
    xe_d = nc.declare_dram_parameter("xe", [HALF, WIDTH], bf16, isOutput=False)
    xo_d = nc.declare_dram_parameter("xo", [HALF, WIDTH], bf16, isOutput=False)
    # one byte-packed param per chunk: [idxe | idxo | S | xsh] along free dim,
    # so each chunk's deps arrive with ONE dma (dispatch cost on sync engine
    # is ~0.65us per dma_start, so fewer+bigger is critical)
    u8 = mybir.dt.uint8
    con_d, con_off = [], []
    for ci, (blocks, ne, no) in enumerate(chunks):
        nt = sum(TL[b] + TH[b] for b in blocks)
        off_ie = 0
        off_io = off_ie + (16 * ne if ci > 0 else 0)
        off_s = off_io + (16 * no if ci > 0 else 0)
        off_x = off_s + P * nt
        end = off_x + 2 * WIDTH * len(blocks)
        con_d.append(nc.declare_dram_parameter(f"c{ci}", [P, end], u8, isOutput=False))
        con_off.append((off_ie, off_io, off_s, off_x))
    # chunk 0's idx as separate params loaded first: first gather starts sooner
    ie0_d = nc.declare_dram_parameter("ie0", [P, 8 * chunks[0][1]], i16, isOutput=False)
    io0_d = nc.declare_dram_parameter("io0", [P, 8 * chunks[0][2]], i16, isOutput=False)
    idt_d = nc.declare_dram_parameter("idt", [P, 2 * P], f8, isOutput=False)
    wt_d = nc.declare_dram_parameter("wt", [P, 2 * (WIDTH + 1)], bf16, isOutput=False)
    brow_d = nc.declare_dram_parameter("brow", [1, WIDTH + 1], bf16, isOutput=False)
    sqd_d = nc.declare_dram_parameter("sqdeg", [1, N_BLOCKS * P], bf16, isOutput=False)
    if generic_affine:
        gb_d = nc.declare_dram_parameter("gb", [P, 2 * WIDTH], f32, isOutput=False)
    # bf16 output (cast back to f32 on the host): halves store traffic
    out_d = nc.declare_dram_parameter("out", [NODES_PER_CORE, WIDTH], bf16, isOutput=True)

    with tile.TileContext(nc) as tc:
        with ExitStack() as ctx:
            const = ctx.enter_context(tc.tile_pool(name="const", bufs=1))
            gpool = ctx.enter_context(tc.tile_pool(name="g", bufs=5))
            apool = ctx.enter_context(tc.tile_pool(name="aggT", bufs=3))
            ypool = ctx.enter_context(tc.tile_pool(name="y", bufs=4))
            stat = ctx.enter_context(tc.tile_pool(name="stat", bufs=6))
            ppool = ctx.enter_context(tc.tile_pool(name="psA", bufs=2, space="PSUM"))
            opsum = ctx.enter_context(tc.tile_pool(name="psO", bufs=2, space="PSUM"))

            # chunk 0's idx first (gates the first gather), then tiny consts
            ie0_sb = const.tile([P, 8 * chunks[0][1]], i16)
            nc.sync.dma_start(ie0_sb[:], ie0_d[:, :])
            io0_sb = const.tile([P, 8 * chunks[0][2]], i16)
            nc.sync.dma_start(io0_sb[:], io0_d[:, :])
            idt_sb = const.tile([P, 2 * P], f8)
            nc.sync.dma_start(idt_sb[:], idt_d[:, :])
            wt_sb = const.tile([P, 2 * (WIDTH + 1)], bf16)
            nc.sync.dma_start(wt_sb[:], wt_d[:, :])
            brow_sb = const.tile([1, WIDTH + 1], bf16)
            nc.sync.dma_start(brow_sb[:], brow_d[:, :])
            sqd_sb = const.tile([1, N_BLOCKS * P], bf16)
            nc.sync.dma_start(sqd_sb[:], sqd_d[:, :])
            con_sb = []
            for ci, (blocks, ne, no) in enumerate(chunks):
                tcon = const.tile([P, con_d[ci].shape[1]], u8, tag=f"c{ci}")
                nc.sync.dma_start(tcon[:], con_d[ci][:, :])
                con_sb.append(tcon)

            def idxe_ap(ci):
                if ci == 0:
                    return ie0_sb[:, :]
                o = con_off[ci][0]
                return con_sb[ci][:, o : con_off[ci][1]].bitcast(i16)

            def idxo_ap(ci):
                if ci == 0:
                    return io0_sb[:, :]
                return con_sb[ci][:, con_off[ci][1] : con_off[ci][2]].bitcast(i16)

            def s_ap_of(ci, tg):
                o = con_off[ci][2] + tg * P
                return con_sb[ci][:, o : o + P].bitcast(f8)

            def xsh_ap(ci, bl, h):
                o = con_off[ci][3] + bl * 2 * WIDTH + h * 2 * P
                return con_sb[ci][:, o : o + 2 * P].bitcast(bf16)
            if generic_affine:
                gb_sb = const.tile([P, 2 * WIDTH], f32)
                nc.sync.dma_start(gb_sb[:], gb_d[:, :])
                gamma_sb = gb_sb[:, :WIDTH]
                beta_sb = gb_sb[:, WIDTH:]
            eps_sb = const.tile([P, 1], f32)
            nc.vector.memset(eps_sb[:], LN_EPS)

            def emit_tail(b, a0, a1):
                """W-matmul + rank-1 bias + LN/ReLU epilogue + store for b."""
                po = opsum.tile([P, WIDTH + 1], f32, tag="po")
                nc.tensor.matmul(
                    out=po[:], lhsT=a0[:], rhs=wt_sb[:, : WIDTH + 1],
                    start=True, stop=False,
                )
                nc.tensor.matmul(
                    out=po[:], lhsT=a1[:], rhs=wt_sb[:, WIDTH + 1 :],
                    start=False, stop=False,
                )
                nc.tensor.matmul(
                    out=po[:],
                    lhsT=sqd_sb[0:1, b * P : (b + 1) * P],
                    rhs=brow_sb[0:1, :],
                    start=False, stop=True,
                )
                # ---- LN epilogue: po rows are A + sqrt(deg) b ----
                sq = ypool.tile([P, WIDTH], f32, tag="sq")
                ssq = stat.tile([P, 1], f32, tag="ssq")
                nc.scalar.activation(
                    out=sq[:], in_=po[:, :WIDTH], func=Act.Square, accum_out=ssq[:]
                )
                mu = stat.tile([P, 1], f32, tag="mu")
                nc.vector.tensor_scalar(
                    out=mu[:], in0=po[:, WIDTH : WIDTH + 1],
                    scalar1=1.0 / WIDTH, scalar2=None, op0=Alu.mult,
                )
                m2 = stat.tile([P, 1], f32, tag="m2")
                nc.vector.tensor_scalar(
                    out=m2[:], in0=mu[:], scalar1=mu[:, :1], scalar2=None,
                    op0=Alu.mult,
                )
                var = stat.tile([P, 1], f32, tag="var")
                nc.vector.tensor_scalar(
                    out=var[:], in0=ssq[:], scalar1=1.0 / WIDTH,
                    scalar2=m2[:, :1], op0=Alu.mult, op1=Alu.subtract,
                )
                sd = stat.tile([P, 1], f32, tag="sd")
                nc.scalar.activation(
                    out=sd[:], in_=var[:], func=Act.Sqrt, bias=eps_sb[:, :1]
                )
                rstd = stat.tile([P, 1], f32, tag="rstd")
                nc.vector.reciprocal(rstd[:], sd[:])
                mrs = stat.tile([P, 1], f32, tag="mrs")
                nc.vector.tensor_scalar(
                    out=mrs[:], in0=mu[:], scalar1=rstd[:, :1], scalar2=-1.0,
                    op0=Alu.mult, op1=Alu.mult,
                )
                yo = ypool.tile([P, WIDTH], bf16, tag="yo")
                if generic_affine:
                    t1 = ypool.tile([P, WIDTH], f32, tag="t1")
                    nc.scalar.activation(
                        out=t1[:], in_=po[:, :WIDTH], func=Act.Identity,
                        scale=rstd[:, :1], bias=mrs[:, :1],
                    )
                    t2 = ypool.tile([P, WIDTH], f32, tag="t2")
                    nc.vector.tensor_tensor(
                        out=t2[:], in0=t1[:], in1=gamma_sb, op=Alu.mult
                    )
                    t3 = ypool.tile([P, WIDTH], f32, tag="t3")
                    nc.vector.tensor_tensor(
                        out=t3[:], in0=t2[:], in1=beta_sb, op=Alu.add
                    )
                    nc.scalar.activation(out=yo[:], in_=t3[:], func=Act.Relu)
                else:
                    nc.scalar.activation(
                        out=yo[:], in_=po[:, :WIDTH], func=Act.Relu,
                        scale=rstd[:, :1], bias=mrs[:, :1],
                    )
                rows = min(P, NODES_PER_CORE - b * P)
                nc.sync.dma_start(out_d[b * P : b * P + rows, :], yo[:rows, :])

            qn = 0
            pending = None  # (b, a0, a1) awaiting W-matmul + epilogue
            for ci, (blocks, ne, no) in enumerate(chunks):
                e0 = int(EOFF[blocks[0]])
                o0 = int(OOFF[blocks[0]])
                tc0 = int(TOFF[blocks[0]])
                ge = go = None
                if ne:
                    ge = gpool.tile([P, ne, WIDTH], bf16, tag="ge")
                    nc.gpsimd.dma_gather(
                        ge[:], xe_d[:, :],
                        idxe_ap(ci),
                        ne * P, ne * P, WIDTH, queue_num=qn % 4,
                    )
                    qn += 1
                if no:
                    go = gpool.tile([P, no, WIDTH], bf16, tag="go")
                    nc.gpsimd.dma_gather(
                        go[:], xo_d[:, :],
                        idxo_ap(ci),
                        no * P, no * P, WIDTH, queue_num=qn % 4,
                    )
                    qn += 1
                for bl, b in enumerate(blocks):
                    tg0 = int(TOFF[b]) - tc0  # chunk-local S tile offset
                    seq = (
                        [(ge, int(EOFF[b]) - e0 + t, tg0 + t) for t in range(TL[b])]
                        + [
                            (go, int(OOFF[b]) - o0 + t, tg0 + TL[b] + t)
                            for t in range(TH[b])
                        ]
                    )
                    nt = len(seq) + 1  # + self tile
                    ps0 = ppool.tile([P, P], f32, tag="ps0")
                    ps1 = ppool.tile([P, P], f32, tag="ps1")
                    for k, (gt, col, tg) in enumerate(seq):
                        s_ap = s_ap_of(ci, tg)
                        nc.tensor.matmul(
                            out=ps0[:], lhsT=gt[:, col, 0:P], rhs=s_ap,
                            start=(k == 0), stop=False,
                        )
                        nc.tensor.matmul(
                            out=ps1[:], lhsT=gt[:, col, P:WIDTH], rhs=s_ap,
                            start=(k == 0), stop=False,
                        )
                    # self-loop tile: lhsT = contiguous shard rows vs identity
                    idw = 0 if b < N_BLOCKS - 1 else P
                    s_ap = idt_sb[:, idw : idw + P]
                    nc.tensor.matmul(
                        out=ps0[:], lhsT=xsh_ap(ci, bl, 0),
                        rhs=s_ap, start=(nt == 1), stop=True,
                    )
                    nc.tensor.matmul(
                        out=ps1[:], lhsT=xsh_ap(ci, bl, 1),
                        rhs=s_ap, start=(nt == 1), stop=True,
                    )
                    # aggT -> SBUF (cast to bf16) for the W-matmul
                    a0 = apool.tile([P, P], bf16, tag="a0")
                    nc.vector.tensor_copy(a0[:], ps0[:])
                    a1 = apool.tile([P, P], bf16, tag="a1")
                    nc.vector.tensor_copy(a1[:], ps1[:])
                    if pending is not None:
                        emit_tail(*pending)
                    pending = (b, a0, a1)
            emit_tail(*pending)
    return nc


def _pack_inputs(TL, TH, dinv, sqdeg_all, S_all, idxe, idxo, x, W, bias, gamma, beta, generic_affine):
    sTL, sTH = sum(TL), sum(TH)
    TOFF = np.concatenate([[0], np.cumsum(np.asarray(TL) + np.asarray(TH))])
    EOFF = np.concatenate([[0], np.cumsum(TL)])
    OOFF = np.concatenate([[0], np.cumsum(TH)])
    chunks = _chunks(TL, TH)

    xs = (dinv[:, None] * x.astype(np.float64)).astype(_bfnp)
    xe = np.ascontiguousarray(xs[0::2])
    xo = np.ascontiguousarray(xs[1::2])

    WT32 = W.T.astype(np.float32)  # [in, out]
    rs = WT32.sum(axis=1, keepdims=True)  # [256, 1] row sums
    WTe = np.concatenate([WT32, rs], axis=1).astype(_bfnp)  # [256, 257]
    wt = np.ascontiguousarray(np.concatenate([WTe[:P], WTe[P:]], axis=1))
    b32 = bias.astype(np.float32)
    brow = np.concatenate([b32, [b32.sum()]])[None, :].astype(_bfnp)

    # shared self-loop identity tiles: full 128 and last-block 106 rows
    idt = np.zeros((P, 2 * P), _f8np)
    pr = np.arange(P)
    idt[pr, pr] = _f8np(1.0)
    rows_last = NODES_PER_CORE - (N_BLOCKS - 1) * P
    prl = np.arange(rows_last)
    idt[prl, P + prl] = _f8np(1.0)

    if generic_affine:
        gb = np.concatenate(
            [
                np.tile(gamma.astype(np.float32)[None, :], (P, 1)),
                np.tile(beta.astype(np.float32)[None, :], (P, 1)),
            ],
            axis=1,
        )

    in_maps = []
    for c in range(N_CORES):
        lo = c * NODES_PER_CORE
        # contiguous shard rows: xsh[p, b*256+ch] = xs[lo+b*128+p, ch]
        xsh = np.zeros((N_BLOCKS * P, WIDTH), _bfnp)
        xsh[:NODES_PER_CORE] = xs[lo : lo + NODES_PER_CORE]
        xsh = np.ascontiguousarray(
            xsh.reshape(N_BLOCKS, P, WIDTH).transpose(1, 0, 2).reshape(P, -1)
        )
        sq = np.zeros((1, N_BLOCKS * P), _bfnp)
        sq[0, :NODES_PER_CORE] = sqdeg_all[lo : lo + NODES_PER_CORE].astype(_bfnp)
        sq[0, NODES_PER_CORE:] = _bfnp(1.0)
        m = {
            "xe": xe,
            "xo": xo,
            "idt": idt,
            "wt": wt,
            "brow": brow,
            "sqdeg": sq,
        }
        for ci, (blocks, ne, no) in enumerate(chunks):
            e0, o0 = int(EOFF[blocks[0]]), int(OOFF[blocks[0]])
            t0, t1 = int(TOFF[blocks[0]]), int(TOFF[blocks[-1] + 1])
            b0, b1 = blocks[0], blocks[-1] + 1
            parts = []
            if ci == 0:
                m["ie0"] = np.ascontiguousarray(idxe[c][:, : 8 * ne])
                m["io0"] = np.ascontiguousarray(idxo[c][:, : 8 * no])
            else:
                parts += [
                    np.ascontiguousarray(idxe[c][:, 8 * e0 : 8 * (e0 + ne)]),
                    np.ascontiguousarray(idxo[c][:, 8 * o0 : 8 * (o0 + no)]),
                ]
            parts += [
                np.ascontiguousarray(S_all[c, :, t0 * P : t1 * P]),
                np.ascontiguousarray(xsh[:, b0 * WIDTH : b1 * WIDTH]),
            ]
            m[f"c{ci}"] = np.concatenate(
                [p.view(np.uint8).reshape(P, -1) for p in parts], axis=1
            )
        if generic_affine:
            m["gb"] = gb
        in_maps.append(m)
    return in_maps


_PROGRAM_CACHE = {}


def kernel(x, edge_index, W, b, gamma, beta, _run_kwargs=None):
    from concourse.bass_utils import run_bass_kernel_spmd

    x = np.asarray(x)
    W = np.asarray(W)
    bias = np.asarray(b)
    gamma = np.asarray(gamma)
    beta = np.asarray(beta)

    TL, TH, dinv, sqdeg_all, S_all, idxe, idxo = _preprocess(edge_index)
    generic_affine = not (np.all(gamma == 1.0) and np.all(beta == 0.0))

    key = (tuple(TL), tuple(TH), generic_affine)
    if key not in _PROGRAM_CACHE:
        nc = _build_program(TL, TH, generic_affine)
        nc.finalize()
        _PROGRAM_CACHE[key] = nc
    nc = _PROGRAM_CACHE[key]

    in_maps = _pack_inputs(
        TL, TH, dinv, sqdeg_all, S_all, idxe, idxo, x, W, bias, gamma, beta,
        generic_affine,
    )

    kwargs = dict(_run_kwargs or {})
    kwargs.pop("_result", None)
    rr = run_bass_kernel_spmd(nc, in_maps, list(range(N_CORES)), **kwargs)
    out = np.concatenate(
        [np.asarray(rr.results[c]["out"]) for c in range(N_CORES)], axis=0
    )
    if _run_kwargs is not None:
        _run_kwargs["_result"] = rr
    return np.ascontiguousarray(out.astype(np.float32))


# revision 59
# speedup vs baseline: 1.7091x; 1.7091x over previous
"""GCN block (GCNConv + LayerNorm + ReLU) on 8 Trainium2 NeuronCores.

Strategy:
  - out = LN(A_norm @ x @ W^T + b) with A_norm = D^-1/2 A D^-1/2 (self-loops
    included).  LayerNorm is scale-invariant per row, so the dst-side scaling
    dinv[dst] can be dropped entirely if the bias is pre-scaled per row:
    LN(dinv_d * (A_d + sqrt(deg_d) * b)) == LN(A_d + sqrt(deg_d) * b), where
    A_d = sum_{e->d} dinv[src] x[src] + dinv[d] x[d].
  - dinv[src] is folded into the gather table on the host (xs = dinv * x,
    bf16), which makes every scatter matrix PURELY BINARY -> precomputed on
    the host and streamed to SBUF as fp8 (0/1 exact): no on-device scatter
    matrix construction at all.
  - Destination nodes sharded contiguously across 8 cores (6250 rows each,
    49 blocks of 128).  Edges bucketed per (dst-block, gather table) and
    padded to 128-edge tiles; source rows fetched with dma_gather (bf16;
    1024 idxs per call is a HW limit; descriptor generation is serial on the
    gpsimd engine and is this kernel's bottleneck at ~2.7 ns/row).  The two
    tables OVERLAP (A = xs[0:32768], B = xs[17232:50000], int16 idx limit):
    overlap-region sources may use either table, so per-core quotas pack
    each block's buckets into the minimal ceil(total/128) tiles instead of
    ceil(A/128)+ceil(B/128) — ~12% fewer gathered rows and matmuls.
  - Per 128-edge tile: aggT += G^T @ S accumulated in PSUM ([ch, dst]); the
    self-loop contribution comes from a contiguous shard load used as lhsT
    against a shared identity tile (keeps 6250 rows/core out of the random
    gather).  aggT @ W^T plus a rank-1 bias matmul (sqrtdeg x [b|sum b])
    yields po = A + sqrt(deg) b with a free row-sum column for the LN mean.
  - Epilogue: Square+accum (ACT) for ssq, small DVE ops for mu/var/rstd,
    one fused ACT Relu(po * rstd + (-mu * rstd)) pass, bf16 store (cast to
    f32 on the host).
  - Per-chunk byte-packed const streams (idx+S+xsh in ONE dma each): each
    dma_start costs ~0.65us of dispatch time on its issuing engine, and a
    coarse const tile stalls every consumer until its whole dma lands, so
    few-but-chunk-aligned loads are critical.  Out-stores ride the sync
    engine after the ~30 load dispatches are done; the first two chunks are
    single blocks so the pipeline fills early.
  - Emission is software-pipelined: block b's aggregation matmuls are
    emitted before block b-1's W-matmul so the tensor engine never waits
    on the PSUM->SBUF copies.
"""

import math
import sys

sys.path.insert(0, "/opt/trn_rl_repo")

import numpy as np
import ml_dtypes

N_NODES = 50000
WIDTH = 256
N_CORES = 8
NODES_PER_CORE = N_NODES // N_CORES  # 6250
P = 128
N_BLOCKS = math.ceil(NODES_PER_CORE / P)  # 49 (last block has 106 rows)
LN_EPS = 1e-5
# Two OVERLAPPING gather tables (int16 idx limit): A = xs[0:32768],
# B = xs[17232:50000].  Sources in [17232, 32768) may use either table, which
# lets every (core, block) bucket pair pack to ceil(total/128) tiles — the
# parity-split ceil waste (~12% of gather rows) disappears.
TBL = 32768  # rows per gather table
CUT = N_NODES - TBL  # 17232: table B starts here

GATHER_TILE_CAP = 8  # max tiles (128 idxs each) per dma_gather call (HW limit 1024 idxs)

_f8np = ml_dtypes.float8_e4m3
_bfnp = ml_dtypes.bfloat16


def _preprocess(edge_index):
    """Bucket non-self-loop edges by (core, dst-block, gather table), pad to
    whole 128-edge tiles, and build the binary scatter-tile stream.

    Table assignment: sources < CUT must use table A, sources >= TBL must use
    table B, and the overlap region may use either — per-core quotas pack
    bucket A to its static tile budget so A+B needs only ceil(total/128)
    tiles.  Per-block processing tile order: A tiles, B tiles, then one
    self-loop identity tile (lhsT = contiguous shard rows).
    """
    src = np.asarray(edge_index[0]).astype(np.int64)
    dst = np.asarray(edge_index[1]).astype(np.int64)

    deg = np.bincount(dst, minlength=N_NODES).astype(np.float64) + 1.0  # + self
    dinv = 1.0 / np.sqrt(deg)
    sqdeg_all = np.sqrt(deg)

    core = dst // NODES_PER_CORE
    r = dst % NODES_PER_CORE
    blk = np.minimum(r // P, N_BLOCKS - 1)
    dcol = r - blk * P
    g = core * N_BLOCKS + blk
    NG = N_CORES * N_BLOCKS

    forcedB = src >= TBL
    flex = (src >= CUT) & ~forcedB
    n_cb = np.bincount(g, minlength=NG).reshape(N_CORES, N_BLOCKS)
    fA_cb = np.bincount(g[src < CUT], minlength=NG).reshape(N_CORES, N_BLOCKS)
    fB_cb = np.bincount(g[forcedB], minlength=NG).reshape(N_CORES, N_BLOCKS)
    fx_cb = n_cb - fA_cb - fB_cb
    TT = np.ceil(n_cb.max(0) / P).astype(np.int64)
    TA = np.ceil(fA_cb.max(0) / P).astype(np.int64)
    TBt = TT - TA
    need_b = np.ceil(fB_cb.max(0) / P).astype(np.int64)
    bump = np.maximum(need_b - TBt, 0)  # (never hit for this graph)
    TT += bump
    TBt += bump
    # per-core rows sent to table A for each block
    aq = np.minimum(fA_cb + fx_cb, P * TA[None, :])
    aq = np.maximum(aq, n_cb - P * TBt[None, :])
    flex_to_A = aq - fA_cb

    # rank flex edges within each (core, block) group -> table choice
    o = np.argsort(g, kind="stable")
    gs, fls = g[o], flex[o].astype(np.int64)
    cum = np.cumsum(fls)
    first = np.r_[0, np.flatnonzero(np.diff(gs)) + 1]
    start_vals = np.zeros(len(first), np.int64)
    start_vals[1:] = cum[first[1:] - 1]
    grp = np.searchsorted(first, np.arange(len(gs)), side="right") - 1
    flexrank = cum - fls - start_vals[grp]  # flex edges before this one
    tab_s = np.where(
        forcedB[o], 1, np.where(fls > 0, flexrank >= flex_to_A.ravel()[gs], 0)
    )
    tab = np.empty(len(src), np.int64)
    tab[o] = tab_s

    gbin = g * 2 + tab
    order = np.argsort(gbin, kind="stable")
    src_s, dcol_s, gbin_s = src[order], dcol[order], gbin[order]

    cnt = np.bincount(gbin, minlength=NG * 2).reshape(N_CORES, N_BLOCKS, 2)
    TL = [int(TA[b]) for b in range(N_BLOCKS)]
    TH = [int(TBt[b]) for b in range(N_BLOCKS)]
    assert (cnt[:, :, 0] <= P * np.asarray(TL)[None, :]).all()
    assert (cnt[:, :, 1] <= P * np.asarray(TH)[None, :]).all()
    sTL, sTH = sum(TL), sum(TH)
    stot = sTL + sTH
    EOFF = np.concatenate([[0], np.cumsum(TL)])  # table-A gather tile offsets
    OOFF = np.concatenate([[0], np.cumsum(TH)])  # table-B gather tile offsets
    TOFF = np.concatenate(
        [[0], np.cumsum(np.asarray(TL) + np.asarray(TH))]
    )  # S-stream tile offsets (self tiles are shared identity tiles)

    starts = np.concatenate([[0], np.cumsum(cnt.ravel())])[:-1]
    j = np.arange(len(gbin_s)) - starts[gbin_s]  # index within bucket
    c = gbin_s // (N_BLOCKS * 2)
    b = (gbin_s // 2) % N_BLOCKS
    t = gbin_s & 1
    til = j // P
    p = j % P

    # gather index arrays (per-table tile order)
    idxe_flat = np.zeros((N_CORES, sTL * P), np.int16)
    idxo_flat = np.zeros((N_CORES, sTH * P), np.int16)
    idx16 = np.where(t == 0, src_s, src_s - CUT).astype(np.int16)
    Je = (EOFF[b] + til) * P + p
    Jo = (OOFF[b] + til) * P + p
    ev = t == 0
    idxe_flat[c[ev], Je[ev]] = idx16[ev]
    idxo_flat[c[~ev], Jo[~ev]] = idx16[~ev]

    # binary S stream: [core, P, stot*P] fp8
    S_all = np.zeros((N_CORES, P, stot * P), _f8np)
    tg = np.where(t == 0, TOFF[b] + til, TOFF[b] + np.asarray(TL)[b] + til)
    S_all[c, p, tg * P + dcol_s] = _f8np(1.0)

    # wrap: flat j -> (partition j%16, column j//16), replicated on 8 stripes
    def wrap(flat, ntiles):
        if ntiles == 0:
            return np.zeros((N_CORES, P, 0), np.int16)
        a = flat.reshape(N_CORES, ntiles * 8, 16).transpose(0, 2, 1)
        return np.ascontiguousarray(np.tile(a, (1, 8, 1)))  # [8,128,cols]

    return (
        TL,
        TH,
        dinv,
        sqdeg_all,
        S_all,
        wrap(idxe_flat, sTL),
        wrap(idxo_flat, sTH),
    )


def _chunks(TL, TH):
    """Group consecutive blocks into gather chunks where EACH table's tile
    count stays within one dma_gather call's limit.  The first two chunks are
    single blocks so the pipeline starts as early as possible."""
    out = []
    cur = []
    ne = no = 0
    for b in range(N_BLOCKS):
        if cur and (
            len(out) < 2
            or ne + TL[b] > GATHER_TILE_CAP
            or no + TH[b] > GATHER_TILE_CAP
        ):
            out.append((cur, ne, no))
            cur, ne, no = [], 0, 0
        cur.append(b)
        ne += TL[b]
        no += TH[b]
    if cur:
        out.append((cur, ne, no))
    return out


def _build_program(TL, TH, generic_affine):
    import concourse.bass as bass
    import concourse.tile as tile
    from concourse import bacc as bacc_mod
    from concourse import mybir
    from contextlib import ExitStack

    f32 = mybir.dt.float32
    bf16 = mybir.dt.bfloat16
    f8 = mybir.dt.float8e4
    i16 = mybir.dt.int16
    Alu = mybir.AluOpType
    Act = mybir.ActivationFunctionType

    sTL, sTH = sum(TL), sum(TH)
    EOFF = np.concatenate([[0], np.cumsum(TL)])
    OOFF = np.concatenate([[0], np.cumsum(TH)])
    TOFF = np.concatenate([[0], np.cumsum(np.asarray(TL) + np.asarray(TH))])
    chunks = _chunks(TL, TH)

    nc = bacc_mod.Bacc(None, target_bir_lowering=False, debug=False, num_swdge_queues=4)
    xe_d = nc.declare_dram_parameter("xa", [TBL, WIDTH], bf16, isOutput=False)
    xo_d = nc.declare_dram_parameter("xb", [TBL, WIDTH], bf16, isOutput=False)
    # one byte-packed param per chunk: [idxe | idxo | S | xsh] along free dim,
    # so each chunk's deps arrive with ONE dma (dispatch cost on sync engine
    # is ~0.65us per dma_start, so fewer+bigger is critical)
    u8 = mybir.dt.uint8
    con_d, con_off = [], []
    for ci, (blocks, ne, no) in enumerate(chunks):
        nt = sum(TL[b] + TH[b] for b in blocks)
        off_ie = 0
        off_io = off_ie + (16 * ne if ci > 0 else 0)
        off_s = off_io + (16 * no if ci > 0 else 0)
        off_x = off_s + P * nt
        end = off_x + 2 * WIDTH * len(blocks)
        con_d.append(nc.declare_dram_parameter(f"c{ci}", [P, end], u8, isOutput=False))
        con_off.append((off_ie, off_io, off_s, off_x))
    # chunk 0's idx as separate params loaded first: first gather starts sooner
    ie0_d = nc.declare_dram_parameter("ie0", [P, 8 * chunks[0][1]], i16, isOutput=False)
    io0_d = nc.declare_dram_parameter("io0", [P, 8 * chunks[0][2]], i16, isOutput=False)
    idt_d = nc.declare_dram_parameter("idt", [P, 2 * P], f8, isOutput=False)
    wt_d = nc.declare_dram_parameter("wt", [P, 2 * (WIDTH + 1)], bf16, isOutput=False)
    brow_d = nc.declare_dram_parameter("brow", [1, WIDTH + 1], bf16, isOutput=False)
    sqd_d = nc.declare_dram_parameter("sqdeg", [1, N_BLOCKS * P], bf16, isOutput=False)
    if generic_affine:
        gb_d = nc.declare_dram_parameter("gb", [P, 2 * WIDTH], f32, isOutput=False)
    # bf16 output (cast back to f32 on the host): halves store traffic
    out_d = nc.declare_dram_parameter("out", [NODES_PER_CORE, WIDTH], bf16, isOutput=True)

    with tile.TileContext(nc) as tc:
        with ExitStack() as ctx:
            const = ctx.enter_context(tc.tile_pool(name="const", bufs=1))
            gpool = ctx.enter_context(tc.tile_pool(name="g", bufs=5))
            apool = ctx.enter_context(tc.tile_pool(name="aggT", bufs=3))
            ypool = ctx.enter_context(tc.tile_pool(name="y", bufs=4))
            stat = ctx.enter_context(tc.tile_pool(name="stat", bufs=6))
            ppool = ctx.enter_context(tc.tile_pool(name="psA", bufs=2, space="PSUM"))
            opsum = ctx.enter_context(tc.tile_pool(name="psO", bufs=2, space="PSUM"))

            # chunk 0's idx first (gates the first gather), then tiny consts
            ie0_sb = const.tile([P, 8 * chunks[0][1]], i16)
            nc.sync.dma_start(ie0_sb[:], ie0_d[:, :])
            io0_sb = const.tile([P, 8 * chunks[0][2]], i16)
            nc.sync.dma_start(io0_sb[:], io0_d[:, :])
            idt_sb = const.tile([P, 2 * P], f8)
            nc.sync.dma_start(idt_sb[:], idt_d[:, :])
            wt_sb = const.tile([P, 2 * (WIDTH + 1)], bf16)
            nc.sync.dma_start(wt_sb[:], wt_d[:, :])
            brow_sb = const.tile([1, WIDTH + 1], bf16)
            nc.sync.dma_start(brow_sb[:], brow_d[:, :])
            sqd_sb = const.tile([1, N_BLOCKS * P], bf16)
            nc.sync.dma_start(sqd_sb[:], sqd_d[:, :])
            con_sb = []
            for ci, (blocks, ne, no) in enumerate(chunks):
                tcon = const.tile([P, con_d[ci].shape[1]], u8, tag=f"c{ci}")
                nc.sync.dma_start(tcon[:], con_d[ci][:, :])
                con_sb.append(tcon)

            def idxe_ap(ci):
                if ci == 0:
                    return ie0_sb[:, :]
                o = con_off[ci][0]
                return con_sb[ci][:, o : con_off[ci][1]].bitcast(i16)

            def idxo_ap(ci):
                if ci == 0:
                    return io0_sb[:, :]
                return con_sb[ci][:, con_off[ci][1] : con_off[ci][2]].bitcast(i16)

            def s_ap_of(ci, tg):
                o = con_off[ci][2] + tg * P
                return con_sb[ci][:, o : o + P].bitcast(f8)

            def xsh_ap(ci, bl, h):
                o = con_off[ci][3] + bl * 2 * WIDTH + h * 2 * P
                return con_sb[ci][:, o : o + 2 * P].bitcast(bf16)
            if generic_affine:
                gb_sb = const.tile([P, 2 * WIDTH], f32)
                nc.sync.dma_start(gb_sb[:], gb_d[:, :])
                gamma_sb = gb_sb[:, :WIDTH]
                beta_sb = gb_sb[:, WIDTH:]
            eps_sb = const.tile([P, 1], f32)
            nc.vector.memset(eps_sb[:], LN_EPS)

            def emit_tail(b, a0, a1):
                """W-matmul + rank-1 bias + LN/ReLU epilogue + store for b."""
                po = opsum.tile([P, WIDTH + 1], f32, tag="po")
                nc.tensor.matmul(
                    out=po[:], lhsT=a0[:], rhs=wt_sb[:, : WIDTH + 1],
                    start=True, stop=False,
                )
                nc.tensor.matmul(
                    out=po[:], lhsT=a1[:], rhs=wt_sb[:, WIDTH + 1 :],
                    start=False, stop=False,
                )
                nc.tensor.matmul(
                    out=po[:],
                    lhsT=sqd_sb[0:1, b * P : (b + 1) * P],
                    rhs=brow_sb[0:1, :],
                    start=False, stop=True,
                )
                # ---- LN epilogue: po rows are A + sqrt(deg) b ----
                sq = ypool.tile([P, WIDTH], f32, tag="sq")
                ssq = stat.tile([P, 1], f32, tag="ssq")
                nc.scalar.activation(
                    out=sq[:], in_=po[:, :WIDTH], func=Act.Square, accum_out=ssq[:]
                )
                mu = stat.tile([P, 1], f32, tag="mu")
                nc.vector.tensor_scalar(
                    out=mu[:], in0=po[:, WIDTH : WIDTH + 1],
                    scalar1=1.0 / WIDTH, scalar2=None, op0=Alu.mult,
                )
                m2 = stat.tile([P, 1], f32, tag="m2")
                nc.vector.tensor_scalar(
                    out=m2[:], in0=mu[:], scalar1=mu[:, :1], scalar2=None,
                    op0=Alu.mult,
                )
                var = stat.tile([P, 1], f32, tag="var")
                nc.vector.tensor_scalar(
                    out=var[:], in0=ssq[:], scalar1=1.0 / WIDTH,
                    scalar2=m2[:, :1], op0=Alu.mult, op1=Alu.subtract,
                )
                sd = stat.tile([P, 1], f32, tag="sd")
                nc.scalar.activation(
                    out=sd[:], in_=var[:], func=Act.Sqrt, bias=eps_sb[:, :1]
                )
                rstd = stat.tile([P, 1], f32, tag="rstd")
                nc.vector.reciprocal(rstd[:], sd[:])
                mrs = stat.tile([P, 1], f32, tag="mrs")
                nc.vector.tensor_scalar(
                    out=mrs[:], in0=mu[:], scalar1=rstd[:, :1], scalar2=-1.0,
                    op0=Alu.mult, op1=Alu.mult,
                )
                yo = ypool.tile([P, WIDTH], bf16, tag="yo")
                if generic_affine:
                    t1 = ypool.tile([P, WIDTH], f32, tag="t1")
                    nc.scalar.activation(
                        out=t1[:], in_=po[:, :WIDTH], func=Act.Identity,
                        scale=rstd[:, :1], bias=mrs[:, :1],
                    )
                    t2 = ypool.tile([P, WIDTH], f32, tag="t2")
                    nc.vector.tensor_tensor(
                        out=t2[:], in0=t1[:], in1=gamma_sb, op=Alu.mult
                    )
                    t3 = ypool.tile([P, WIDTH], f32, tag="t3")
                    nc.vector.tensor_tensor(
                        out=t3[:], in0=t2[:], in1=beta_sb, op=Alu.add
                    )
                    nc.scalar.activation(out=yo[:], in_=t3[:], func=Act.Relu)
                else:
                    nc.scalar.activation(
                        out=yo[:], in_=po[:, :WIDTH], func=Act.Relu,
                        scale=rstd[:, :1], bias=mrs[:, :1],
                    )
                rows = min(P, NODES_PER_CORE - b * P)
                nc.sync.dma_start(out_d[b * P : b * P + rows, :], yo[:rows, :])

            qn = 0
            pending = None  # (b, a0, a1) awaiting W-matmul + epilogue
            for ci, (blocks, ne, no) in enumerate(chunks):
                e0 = int(EOFF[blocks[0]])
                o0 = int(OOFF[blocks[0]])
                tc0 = int(TOFF[blocks[0]])
                ge = go = None
                if ne:
                    ge = gpool.tile([P, ne, WIDTH], bf16, tag="ge")
                    nc.gpsimd.dma_gather(
                        ge[:], xe_d[:, :],
                        idxe_ap(ci),
                        ne * P, ne * P, WIDTH, queue_num=qn % 4,
                    )
                    qn += 1
                if no:
                    go = gpool.tile([P, no, WIDTH], bf16, tag="go")
                    nc.gpsimd.dma_gather(
                        go[:], xo_d[:, :],
                        idxo_ap(ci),
                        no * P, no * P, WIDTH, queue_num=qn % 4,
                    )
                    qn += 1
                for bl, b in enumerate(blocks):
                    tg0 = int(TOFF[b]) - tc0  # chunk-local S tile offset
                    seq = (
                        [(ge, int(EOFF[b]) - e0 + t, tg0 + t) for t in range(TL[b])]
                        + [
                            (go, int(OOFF[b]) - o0 + t, tg0 + TL[b] + t)
                            for t in range(TH[b])
                        ]
                    )
                    nt = len(seq) + 1  # + self tile
                    ps0 = ppool.tile([P, P], f32, tag="ps0")
                    ps1 = ppool.tile([P, P], f32, tag="ps1")
                    for k, (gt, col, tg) in enumerate(seq):
                        s_ap = s_ap_of(ci, tg)
                        nc.tensor.matmul(
                            out=ps0[:], lhsT=gt[:, col, 0:P], rhs=s_ap,
                            start=(k == 0), stop=False,
                        )
                        nc.tensor.matmul(
                            out=ps1[:], lhsT=gt[:, col, P:WIDTH], rhs=s_ap,
                            start=(k == 0), stop=False,
                        )
                    # self-loop tile: lhsT = contiguous shard rows vs identity
                    idw = 0 if b < N_BLOCKS - 1 else P
                    s_ap = idt_sb[:, idw : idw + P]
                    nc.tensor.matmul(
                        out=ps0[:], lhsT=xsh_ap(ci, bl, 0),
                        rhs=s_ap, start=(nt == 1), stop=True,
                    )
                    nc.tensor.matmul(
                        out=ps1[:], lhsT=xsh_ap(ci, bl, 1),
                        rhs=s_ap, start=(nt == 1), stop=True,
                    )
                    # aggT -> SBUF (cast to bf16) for the W-matmul
                    a0 = apool.tile([P, P], bf16, tag="a0")
                    nc.vector.tensor_copy(a0[:], ps0[:])
                    a1 = apool.tile([P, P], bf16, tag="a1")
                    nc.vector.tensor_copy(a1[:], ps1[:])
                    if pending is not None:
                        emit_tail(*pending)
                    pending = (b, a0, a1)
            emit_tail(*pending)
    return nc


def _pack_inputs(TL, TH, dinv, sqdeg_all, S_all, idxe, idxo, x, W, bias, gamma, beta, generic_affine):
    sTL, sTH = sum(TL), sum(TH)
    TOFF = np.concatenate([[0], np.cumsum(np.asarray(TL) + np.asarray(TH))])
    EOFF = np.concatenate([[0], np.cumsum(TL)])
    OOFF = np.concatenate([[0], np.cumsum(TH)])
    chunks = _chunks(TL, TH)

    xs = (dinv[:, None] * x.astype(np.float64)).astype(_bfnp)
    xa = np.ascontiguousarray(xs[:TBL])
    xb = np.ascontiguousarray(xs[CUT:])

    WT32 = W.T.astype(np.float32)  # [in, out]
    rs = WT32.sum(axis=1, keepdims=True)  # [256, 1] row sums
    WTe = np.concatenate([WT32, rs], axis=1).astype(_bfnp)  # [256, 257]
    wt = np.ascontiguousarray(np.concatenate([WTe[:P], WTe[P:]], axis=1))
    b32 = bias.astype(np.float32)
    brow = np.concatenate([b32, [b32.sum()]])[None, :].astype(_bfnp)

    # shared self-loop identity tiles: full 128 and last-block 106 rows
    idt = np.zeros((P, 2 * P), _f8np)
    pr = np.arange(P)
    idt[pr, pr] = _f8np(1.0)
    rows_last = NODES_PER_CORE - (N_BLOCKS - 1) * P
    prl = np.arange(rows_last)
    idt[prl, P + prl] = _f8np(1.0)

    if generic_affine:
        gb = np.concatenate(
            [
                np.tile(gamma.astype(np.float32)[None, :], (P, 1)),
                np.tile(beta.astype(np.float32)[None, :], (P, 1)),
            ],
            axis=1,
        )

    in_maps = []
    for c in range(N_CORES):
        lo = c * NODES_PER_CORE
        # contiguous shard rows: xsh[p, b*256+ch] = xs[lo+b*128+p, ch]
        xsh = np.zeros((N_BLOCKS * P, WIDTH), _bfnp)
        xsh[:NODES_PER_CORE] = xs[lo : lo + NODES_PER_CORE]
        xsh = np.ascontiguousarray(
            xsh.reshape(N_BLOCKS, P, WIDTH).transpose(1, 0, 2).reshape(P, -1)
        )
        sq = np.zeros((1, N_BLOCKS * P), _bfnp)
        sq[0, :NODES_PER_CORE] = sqdeg_all[lo : lo + NODES_PER_CORE].astype(_bfnp)
        sq[0, NODES_PER_CORE:] = _bfnp(1.0)
        m = {
            "xa": xa,
            "xb": xb,
            "idt": idt,
            "wt": wt,
            "brow": brow,
            "sqdeg": sq,
        }
        for ci, (blocks, ne, no) in enumerate(chunks):
            e0, o0 = int(EOFF[blocks[0]]), int(OOFF[blocks[0]])
            t0, t1 = int(TOFF[blocks[0]]), int(TOFF[blocks[-1] + 1])
            b0, b1 = blocks[0], blocks[-1] + 1
            parts = []
            if ci == 0:
                m["ie0"] = np.ascontiguousarray(idxe[c][:, : 8 * ne])
                m["io0"] = np.ascontiguousarray(idxo[c][:, : 8 * no])
            else:
                parts += [
                    np.ascontiguousarray(idxe[c][:, 8 * e0 : 8 * (e0 + ne)]),
                    np.ascontiguousarray(idxo[c][:, 8 * o0 : 8 * (o0 + no)]),
                ]
            parts += [
                np.ascontiguousarray(S_all[c, :, t0 * P : t1 * P]),
                np.ascontiguousarray(xsh[:, b0 * WIDTH : b1 * WIDTH]),
            ]
            m[f"c{ci}"] = np.concatenate(
                [p.view(np.uint8).reshape(P, -1) for p in parts], axis=1
            )
        if generic_affine:
            m["gb"] = gb
        in_maps.append(m)
    return in_maps


_PROGRAM_CACHE = {}


def kernel(x, edge_index, W, b, gamma, beta, _run_kwargs=None):
    from concourse.bass_utils import run_bass_kernel_spmd

    x = np.asarray(x)
    W = np.asarray(W)
    bias = np.asarray(b)
    gamma = np.asarray(gamma)
    beta = np.asarray(beta)

    TL, TH, dinv, sqdeg_all, S_all, idxe, idxo = _preprocess(edge_index)
    generic_affine = not (np.all(gamma == 1.0) and np.all(beta == 0.0))

    key = (tuple(TL), tuple(TH), generic_affine)
    if key not in _PROGRAM_CACHE:
        nc = _build_program(TL, TH, generic_affine)
        nc.finalize()
        _PROGRAM_CACHE[key] = nc
    nc = _PROGRAM_CACHE[key]

    in_maps = _pack_inputs(
        TL, TH, dinv, sqdeg_all, S_all, idxe, idxo, x, W, bias, gamma, beta,
        generic_affine,
    )

    kwargs = dict(_run_kwargs or {})
    kwargs.pop("_result", None)
    rr = run_bass_kernel_spmd(nc, in_maps, list(range(N_CORES)), **kwargs)
    out = np.concatenate(
        [np.asarray(rr.results[c]["out"]) for c in range(N_CORES)], axis=0
    )
    if _run_kwargs is not None:
        _run_kwargs["_result"] = rr
    return np.ascontiguousarray(out.astype(np.float32))
